# revision 19
# baseline (speedup 1.0000x reference)
"""AnchorPatchPooling Trainium2 kernel.

Math (per sample n, channel c, part p):
  out[n,c,p] = sum_{k: lab[k]=p} feats[n,c,k]*vm[n,k] / max(count[n,p],1)
             + where(patch_count[p]>0, max(-100, max_{k: lab[k]=p} feats[n,c,k]), 0)

Strategy:
 - Data-parallel over n across 8 cores (4 samples/core), no collectives.
 - part_labels is shared across samples: one host argsort groups the k axis
   into 16 contiguous segments, PADDED to a uniform stride S (feats pad
   -100.0 == the reference's include_self floor, valid_mask pad 0.0 so
   padded products vanish from the sum). Uniform segments turn the 16
   per-part reductions into ONE 3D-AP tensor_reduce per (sample, c-block).
 - bf16 storage/compute halves both DMA and DVE elementwise cost; sums
   accumulate into f32 (rel err ~1e-3, well under the 2e-2 gate).
 - valid_mask enters on-device via a 0-partition-stride DRAM->SBUF DMA
   broadcast, then one in-place bf16 multiply per (sample, c-block).

Per core, per sample s, per channel-block cb (c = 128*cb .. 128*cb+127):
  - DMA feats_pad[s, cb] tile [128, 16*S] bf16
  - DVE: maxs[128,16]  = tensor_reduce_max over [128, 16, S] (raw tile)
  - DVE: tile *= vm_broadcast (in-place, bf16 2x mode)
  - DVE: sums[128,16]  = tensor_reduce_add over [128, 16, S] -> f32
  - combine: out = sums * recip(max(cnt,1)) + maxs
"""

import numpy as np

N, C, K, PARTS = 32, 256, 8192, 16
MAX_INIT = -100.0
NCORES = 8
NLOC = N // NCORES  # samples per core
P = 128
NCB = C // P  # channel blocks per sample

_CACHE = {}
_PATCHED = False


def _patch_bass():
    """This container's walrus build accepts at most ONE sync-wait per
    instruction; Tile's tail drain aggregates several. Split any multi-wait
    instruction into a chain of single-wait Drains at BIR-serialization time
    (covers both compile_bass_kernel and the bass2jax/PJRT hook path)."""
    global _PATCHED
    if _PATCHED:
        return
    import orjson
    import concourse.bass as bass

    orig = bass.Bass.to_json_bytes

    def patched(self):
        d = orjson.loads(orig(self))
        for fn in d.get("functions", []):
            for blk in fn.get("blocks", []):
                out, ctr = [], 0
                for ins in blk["instructions"]:
                    si = ins.get("sync_info") or {}
                    waits = si.get("on_wait") or []
                    if len(waits) > 1:
                        for w in waits[:-1]:
                            ctr += 1
                            out.append({
                                "debug": ins.get("debug"),
                                "engine": ins["engine"],
                                "ins": [],
                                "name": f"{ins['name']}-sw{ctr}",
                                "opcode": "NoOp",
                                "outs": [],
                                "sync_info": {"on_update": [],
                                              "on_wait": [w]},
                            })
                        si["on_wait"] = waits[-1:]
                    out.append(ins)
                blk["instructions"] = out
        return orjson.dumps(d)

    bass.Bass.to_json_bytes = patched
    _PATCHED = True


def _build(S, empty_parts, act_sum_rows=6, gp_mult_rows=4):
    import concourse.bass as bass
    import concourse.tile as tile
    from concourse import mybir

    _patch_bass()

    KP = PARTS * S
    bf = mybir.dt.bfloat16
    f32 = mybir.dt.float32
    nc = bass.Bass()
    feats_e = nc.declare_dram_parameter("feats", [NLOC, C, KP], bf, isOutput=False)
    vm_e = nc.declare_dram_parameter("vm", [NLOC, KP], bf, isOutput=False)
    cnt_e = nc.declare_dram_parameter("cnt", [NLOC, P, PARTS], f32, isOutput=False)
    out_e = nc.declare_dram_parameter("out", [NLOC, NCB, P, PARTS], f32, isOutput=True)

    with tile.TileContext(nc) as tc:
        with tc.tile_pool(name="big", bufs=3) as bigp, \
             tc.tile_pool(name="vmp", bufs=2) as vmp, \
             tc.tile_pool(name="small", bufs=8) as smallp:
            for s in range(NLOC):
                vmb = vmp.tile([P, KP], bf, tag="vmb")
                row_ap = vm_e[s, None, :]
                bcast_ap = bass.AP(
                    tensor=row_ap.tensor,
                    offset=row_ap.offset,
                    ap=[[0, P], row_ap.ap[-1]],
                )
                nc.sync.dma_start(out=vmb[:], in_=bcast_ap)

                cntt = smallp.tile([P, PARTS], f32, tag="cnt")
                nc.sync.dma_start(out=cntt[:], in_=cnt_e[s])
                rec = smallp.tile([P, PARTS], f32, tag="rec")
                nc.vector.tensor_scalar_max(rec[:], cntt[:], 1.0)
                nc.vector.reciprocal(rec[:], rec[:])

                for cb in range(NCB):
                    ft = bigp.tile([P, KP], bf, tag="ft")
                    nc.sync.dma_start(
                        out=ft[:], in_=feats_e[s, cb * P:(cb + 1) * P, :]
                    )
                    scr = bigp.tile([P, (S // 2 + 1) * PARTS], bf, tag="scr")
                    sums = smallp.tile([P, PARTS], f32, tag="sums")
                    maxs = smallp.tile([P, PARTS], f32, tag="maxs")

                    # Layout is rank-major/part-minor (col = rank*16 + p), so
                    # every pairwise fold is a FLAT contiguous 2D slice (the
                    # DVE packs those at full rate). Fold ceil(R/2) per step;
                    # odd R leaves the middle rank in place for next round.
                    def fold_chain(src, dst, op, final_out, in_place):
                        R = S
                        buf = src
                        first = True
                        while R > 1:
                            Hn = R // 2
                            lo = buf[:, 0:Hn * PARTS]
                            hi = buf[:, (R - Hn) * PARTS:R * PARTS]
                            if R == 2:
                                nc.vector.tensor_tensor(
                                    out=final_out, in0=buf[:, 0:PARTS],
                                    in1=buf[:, PARTS:2 * PARTS], op=op)
                            elif first and not in_place:
                                nc.vector.tensor_tensor(
                                    out=dst[:, 0:Hn * PARTS], in0=lo,
                                    in1=hi, op=op)
                                if R % 2:
                                    # carry the untouched middle rank over
                                    nc.vector.tensor_copy(
                                        dst[:, Hn * PARTS:(Hn + 1) * PARTS],
                                        buf[:, Hn * PARTS:(Hn + 1) * PARTS])
                                buf = dst
                            else:
                                nc.vector.tensor_tensor(
                                    out=lo, in0=lo, in1=hi, op=op)
                            first = False
                            R -= Hn

                    row = s * NCB + cb
                    # max over raw feats: first fold lands in scr
                    fold_chain(ft[:], scr[:], mybir.AluOpType.max, maxs[:],
                               in_place=False)
                    # mask fold-in (scheduled after the raw-feats first fold);
                    # some rows run it on the otherwise-idle GpSimd engine
                    gp_rows = [r for r in range(8) if r % 2 == 0][:gp_mult_rows]
                    act_rows = [r for r in range(8) if r % 4 != 3][:act_sum_rows]
                    mul_eng = nc.gpsimd if row in gp_rows else nc.vector
                    mul_eng.tensor_tensor(
                        out=ft[:], in0=ft[:], in1=vmb[:],
                        op=mybir.AluOpType.mult,
                    )
                    # masked sum: DVE fold chain, or per-segment accumulate
                    # on the otherwise-idle ACT engine (strided column reads)
                    if row in act_rows:
                        act_scr = smallp.tile([P, S], bf, tag="actscr")
                        ftr = ft[:].rearrange("p (r g) -> p g r", g=PARTS)
                        for g in range(PARTS):
                            nc.scalar.activation(
                                out=act_scr[:],
                                in_=ftr[:, g, :],
                                func=mybir.ActivationFunctionType.Copy,
                                accum_out=sums[:, g:g + 1],
                            )
                    else:
                        fold_chain(ft[:], ft[:], mybir.AluOpType.add, sums[:],
                                   in_place=True)
                    for p in empty_parts:
                        # empty part: reference yields 0 (patch_count == 0)
                        nc.vector.memset(maxs[:, p:p + 1], 0.0)

                    res = smallp.tile([P, PARTS], f32, tag="res")
                    nc.vector.tensor_tensor(
                        out=res[:], in0=sums[:], in1=rec[:], op=mybir.AluOpType.mult
                    )
                    nc.vector.tensor_tensor(
                        out=res[:], in0=res[:], in1=maxs[:], op=mybir.AluOpType.add
                    )
                    nc.sync.dma_start(out=out_e[s, cb], in_=res[:])
    return nc


def kernel(feats, part_labels, valid_mask, _timing=None):
    import ml_dtypes
    from concourse.bass_utils import run_bass_kernel_spmd

    feats = np.asarray(feats, dtype=np.float32)
    labels = np.asarray(part_labels).astype(np.int64)
    vm = np.asarray(valid_mask).astype(np.float32)

    # ---- host specialization on the (replicated) labels ----
    order = np.argsort(labels, kind="stable")
    seg_len = np.bincount(labels, minlength=PARTS).astype(np.int64)
    seg_off = np.concatenate([[0], np.cumsum(seg_len)[:-1]]).astype(np.int64)
    # rank-major/part-minor layout: col = rank*PARTS + part. Any S works for
    # the ceil-halving fold chain; round up to a multiple of 16 for DMA.
    S = int(-(-int(seg_len.max()) // 16) * 16)
    S = max(S, 16)
    KP = PARTS * S
    ranks = np.arange(K, dtype=np.int64) - np.repeat(seg_off, seg_len)
    dest = ranks * PARTS + np.repeat(np.arange(PARTS, dtype=np.int64), seg_len)

    bf16 = ml_dtypes.bfloat16
    feats_pad = np.full((N, C, KP), MAX_INIT, dtype=bf16)
    feats_pad[:, :, dest] = feats[:, :, order].astype(bf16)
    vm_pad = np.zeros((N, KP), dtype=bf16)
    vm_pad[:, dest] = vm[:, order].astype(bf16)

    counts = np.zeros((N, PARTS), dtype=np.float32)
    np.add.at(counts, (slice(None), labels), vm)
    cnt_b = np.ascontiguousarray(
        np.broadcast_to(counts[:, None, :], (N, P, PARTS)).astype(np.float32)
    )
    empty_parts = [p for p in range(PARTS) if seg_len[p] == 0]

    key = (S, tuple(empty_parts))
    if key not in _CACHE:
        _CACHE[key] = _build(S, empty_parts)
    nc = _CACHE[key]

    in_maps = [
        {
            "feats": feats_pad[i * NLOC:(i + 1) * NLOC],
            "vm": vm_pad[i * NLOC:(i + 1) * NLOC],
            "cnt": cnt_b[i * NLOC:(i + 1) * NLOC],
        }
        for i in range(NCORES)
    ]
    res = run_bass_kernel_spmd(
        nc, in_maps, core_ids=list(range(NCORES)),
        **({} if _timing is None else _timing),
    )
    if _timing is not None:
        kernel.last_result = res
    out = np.concatenate(
        [r["out"].reshape(NLOC, C, PARTS) for r in res.results], axis=0
    )
    return out


# revision 23
# speedup vs baseline: 1.2281x; 1.2281x over previous
"""AnchorPatchPooling Trainium2 kernel.

Math (per sample n, channel c, part p):
  out[n,c,p] = sum_{k: lab[k]=p} feats[n,c,k]*vm[n,k] / max(count[n,p],1)
             + where(patch_count[p]>0, max(-100, max_{k: lab[k]=p} feats[n,c,k]), 0)

Strategy:
 - Data-parallel over n across 8 cores (4 samples/core), no collectives.
 - part_labels is shared across samples: one host argsort groups the k axis
   into 16 contiguous segments, PADDED to a uniform stride S (feats pad
   -100.0 == the reference's include_self floor, valid_mask pad 0.0 so
   padded products vanish from the sum). Uniform segments turn the 16
   per-part reductions into ONE 3D-AP tensor_reduce per (sample, c-block).
 - bf16 storage/compute halves both DMA and DVE elementwise cost; sums
   accumulate into f32 (rel err ~1e-3, well under the 2e-2 gate).
 - valid_mask enters on-device via a 0-partition-stride DRAM->SBUF DMA
   broadcast, then one in-place bf16 multiply per (sample, c-block).

Per core, per sample s, per channel-block cb (c = 128*cb .. 128*cb+127):
  - DMA feats_pad[s, cb] tile [128, 16*S] bf16
  - DVE: maxs[128,16]  = tensor_reduce_max over [128, 16, S] (raw tile)
  - DVE: tile *= vm_broadcast (in-place, bf16 2x mode)
  - DVE: sums[128,16]  = tensor_reduce_add over [128, 16, S] -> f32
  - combine: out = sums * recip(max(cnt,1)) + maxs
"""

import numpy as np

N, C, K, PARTS = 32, 256, 8192, 16
MAX_INIT = -100.0
NCORES = 8
NLOC = N // NCORES  # samples per core
P = 128
NCB = C // P  # channel blocks per sample

_CACHE = {}
_PATCHED = False


def _patch_bass():
    """This container's walrus build accepts at most ONE sync-wait per
    instruction; Tile's tail drain aggregates several. Split any multi-wait
    instruction into a chain of single-wait Drains at BIR-serialization time
    (covers both compile_bass_kernel and the bass2jax/PJRT hook path)."""
    global _PATCHED
    if _PATCHED:
        return
    import orjson
    import concourse.bass as bass

    orig = bass.Bass.to_json_bytes

    def patched(self):
        d = orjson.loads(orig(self))
        for fn in d.get("functions", []):
            for blk in fn.get("blocks", []):
                out, ctr = [], 0
                for ins in blk["instructions"]:
                    si = ins.get("sync_info") or {}
                    waits = si.get("on_wait") or []
                    if len(waits) > 1:
                        for w in waits[:-1]:
                            ctr += 1
                            out.append({
                                "debug": ins.get("debug"),
                                "engine": ins["engine"],
                                "ins": [],
                                "name": f"{ins['name']}-sw{ctr}",
                                "opcode": "NoOp",
                                "outs": [],
                                "sync_info": {"on_update": [],
                                              "on_wait": [w]},
                            })
                        si["on_wait"] = waits[-1:]
                    out.append(ins)
                blk["instructions"] = out
        return orjson.dumps(d)

    bass.Bass.to_json_bytes = patched
    _PATCHED = True


def _build(S, empty_parts, act_sum_rows=0, gp_mult_rows=0):
    import concourse.bass as bass
    import concourse.tile as tile
    from concourse import mybir

    _patch_bass()

    KP = PARTS * S
    bf = mybir.dt.bfloat16
    f32 = mybir.dt.float32
    nc = bass.Bass()
    feats_e = nc.declare_dram_parameter("feats", [NLOC, C, KP], bf, isOutput=False)
    vm_e = nc.declare_dram_parameter("vm", [NLOC, KP], bf, isOutput=False)
    cnt_e = nc.declare_dram_parameter("cnt", [NLOC, P, NCB * PARTS], f32,
                                      isOutput=False)
    out_e = nc.declare_dram_parameter("out", [NLOC, NCB, P, PARTS], f32, isOutput=True)

    GW = NCB * PARTS  # final width: (cb, part) pairs

    with tile.TileContext(nc) as tc:
        with tc.tile_pool(name="big", bufs=3) as bigp, \
             tc.tile_pool(name="vmp", bufs=2) as vmp, \
             tc.tile_pool(name="scrp", bufs=2) as scrp, \
             tc.tile_pool(name="small", bufs=8) as smallp:
            for s in range(NLOC):
                vmb = vmp.tile([P, KP], bf, tag="vmb")
                row_ap = vm_e[s, None, :]
                bcast_ap = bass.AP(
                    tensor=row_ap.tensor,
                    offset=row_ap.offset,
                    ap=[[0, P], row_ap.ap[-1]],
                )
                nc.sync.dma_start(out=vmb[:], in_=bcast_ap)

                cntt = smallp.tile([P, GW], f32, tag="cnt")
                nc.sync.dma_start(out=cntt[:], in_=cnt_e[s])
                rec = smallp.tile([P, GW], f32, tag="rec")
                nc.vector.tensor_scalar_max(rec[:], cntt[:], 1.0)
                nc.vector.reciprocal(rec[:], rec[:])

                # one tile holds BOTH channel blocks: [c(128), cb(2), col(KP)]
                ft = bigp.tile([P, NCB, KP], bf, tag="ft")
                nc.sync.dma_start(
                    out=ft[:],
                    in_=feats_e[s].rearrange("(b c) w -> c b w", c=P),
                )
                scr = bigp.tile([P, NCB, (S // 2 + 1) * PARTS], bf, tag="scr")
                sums = smallp.tile([P, GW], f32, tag="sums")
                maxs = smallp.tile([P, GW], f32, tag="maxs")

                # Layout is rank-major/part-minor (col = rank*16 + p): every
                # fold is a contiguous run per cb. Fold ceil(R/2) per step;
                # odd R leaves the middle rank in place for the next round.
                def fold_chain(src3, dst3, op, final3, in_place):
                    R = S
                    buf = src3
                    first = True
                    while R > 1:
                        Hn = R // 2
                        if R == 2:
                            nc.vector.tensor_tensor(
                                out=final3, in0=buf[:, :, 0:PARTS],
                                in1=buf[:, :, PARTS:2 * PARTS], op=op)
                        elif first and not in_place:
                            nc.vector.tensor_tensor(
                                out=dst3[:, :, 0:Hn * PARTS],
                                in0=buf[:, :, 0:Hn * PARTS],
                                in1=buf[:, :, (R - Hn) * PARTS:R * PARTS],
                                op=op)
                            if R % 2:
                                nc.vector.tensor_copy(
                                    dst3[:, :, Hn * PARTS:(Hn + 1) * PARTS],
                                    buf[:, :, Hn * PARTS:(Hn + 1) * PARTS])
                            buf = dst3
                        else:
                            nc.vector.tensor_tensor(
                                out=buf[:, :, 0:Hn * PARTS],
                                in0=buf[:, :, 0:Hn * PARTS],
                                in1=buf[:, :, (R - Hn) * PARTS:R * PARTS],
                                op=op)
                        first = False
                        R -= Hn

                maxs3 = maxs[:].rearrange("p (b g) -> p b g", b=NCB)
                sums3 = sums[:].rearrange("p (b g) -> p b g", b=NCB)
                # max over raw feats: first fold lands in scr
                fold_chain(ft[:], scr[:], mybir.AluOpType.max, maxs3,
                           in_place=False)
                # mask fold-in (scheduled after the raw-feats first fold);
                # flat [128, KP] per channel block keeps the DVE packed mode
                for cb in range(NCB):
                    nc.vector.tensor_tensor(
                        out=ft[:, cb, :], in0=ft[:, cb, :], in1=vmb[:],
                        op=mybir.AluOpType.mult,
                    )
                # masked sum: fold ft in place
                fold_chain(ft[:], ft[:], mybir.AluOpType.add, sums3,
                           in_place=True)
                for p in empty_parts:
                    # empty part: reference yields 0 (patch_count == 0)
                    nc.vector.memset(maxs[:, p:p + 1], 0.0)
                    nc.vector.memset(maxs[:, PARTS + p:PARTS + p + 1], 0.0)

                res = smallp.tile([P, GW], f32, tag="res")
                nc.vector.tensor_tensor(
                    out=res[:], in0=sums[:], in1=rec[:], op=mybir.AluOpType.mult
                )
                nc.vector.tensor_tensor(
                    out=res[:], in0=res[:], in1=maxs[:], op=mybir.AluOpType.add
                )
                nc.sync.dma_start(
                    out=out_e[s].rearrange("b c g -> c b g"),
                    in_=res[:].rearrange("p (b g) -> p b g", b=NCB),
                )
    return nc


def kernel(feats, part_labels, valid_mask, _timing=None):
    import ml_dtypes
    from concourse.bass_utils import run_bass_kernel_spmd

    feats = np.asarray(feats, dtype=np.float32)
    labels = np.asarray(part_labels).astype(np.int64)
    vm = np.asarray(valid_mask).astype(np.float32)

    # ---- host specialization on the (replicated) labels ----
    order = np.argsort(labels, kind="stable")
    seg_len = np.bincount(labels, minlength=PARTS).astype(np.int64)
    seg_off = np.concatenate([[0], np.cumsum(seg_len)[:-1]]).astype(np.int64)
    # rank-major/part-minor layout: col = rank*PARTS + part. Any S works for
    # the ceil-halving fold chain; round up to a multiple of 16 for DMA.
    S = int(-(-int(seg_len.max()) // 16) * 16)
    S = max(S, 16)
    KP = PARTS * S
    ranks = np.arange(K, dtype=np.int64) - np.repeat(seg_off, seg_len)
    dest = ranks * PARTS + np.repeat(np.arange(PARTS, dtype=np.int64), seg_len)

    bf16 = ml_dtypes.bfloat16
    feats_pad = np.full((N, C, KP), MAX_INIT, dtype=bf16)
    feats_pad[:, :, dest] = feats[:, :, order].astype(bf16)
    vm_pad = np.zeros((N, KP), dtype=bf16)
    vm_pad[:, dest] = vm[:, order].astype(bf16)

    counts = np.zeros((N, PARTS), dtype=np.float32)
    np.add.at(counts, (slice(None), labels), vm)
    # replicate along partitions and duplicate per channel-block: [N, P, 2*16]
    cnt_b = np.ascontiguousarray(
        np.broadcast_to(counts[:, None, None, :], (N, P, NCB, PARTS))
        .reshape(N, P, NCB * PARTS).astype(np.float32)
    )
    empty_parts = [p for p in range(PARTS) if seg_len[p] == 0]

    key = (S, tuple(empty_parts))
    if key not in _CACHE:
        _CACHE[key] = _build(S, empty_parts)
    nc = _CACHE[key]

    in_maps = [
        {
            "feats": feats_pad[i * NLOC:(i + 1) * NLOC],
            "vm": vm_pad[i * NLOC:(i + 1) * NLOC],
            "cnt": cnt_b[i * NLOC:(i + 1) * NLOC],
        }
        for i in range(NCORES)
    ]
    res = run_bass_kernel_spmd(
        nc, in_maps, core_ids=list(range(NCORES)),
        **({} if _timing is None else _timing),
    )
    if _timing is not None:
        kernel.last_result = res
    out = np.concatenate(
        [r["out"].reshape(NLOC, C, PARTS) for r in res.results], axis=0
    )
    return out
